# revision 25
# baseline (speedup 1.0000x reference)
"""BitLinear-1.58 (ternary-quantized linear) Trainium2 Bass kernel.

Math (matches the reference):
    gamma = mean(|W|)                       # global scalar over full W
    Wq    = clip(round(W / (gamma+eps)), -1, 1)   # ternary {-1,0,1}
    out   = x @ Wq.T + b                    # x: [B,S,in] -> [B,S,out]

Sharding: column-parallel over 8 NeuronCores. Each core owns a 512-wide
slice of out_features (its W shard + bias shard), x is replicated.

The mean-|W| reduction is split into two device launches: launch 1
computes per-core partial |W| sums over each core's shard (all 16.7M
element-abs/add work on device); the host combines the 8 partial
vectors into the scalar threshold (the 8-way all-reduce step), which
feeds launch 2. Rationale: a NEFF that contains a collective_compute
executes every matmul at ~263 ns instead of ~216 ns on this runtime (a
~22% PE tax measured on 8-core microbenchmarks, regardless of the
collective's placement or size), which costs far more than the 8-way
scalar combine is worth.

Quantization is done on-device by threshold compare (exactly equivalent
to round+clip for ternary output, incl. the round-half-to-even edge):
    Wq = (W > thr) - (W < -thr),  thr = 0.5*(gamma+eps)
implemented as two DVE ops per W chunk:
    neg = (W < -thr);  Wq = (W > thr) - neg   (scalar_tensor_tensor)

Matmul: x cast to bf16 (host-side, same RNE rounding as on-device), Wq
in bf16 (exact: ternary), PSUM accumulates f32. Per-core GEMM is
[8192 x 4096] @ [4096 x 512] done as 64 m-tiles x 32 k-tiles of
(lhsT=[128k,128m] stationary, rhs=[128k,512n] moving). Bias is added in
f32 during PSUM evacuation on the vector engine.
"""

from contextlib import ExitStack

import numpy as np
import ml_dtypes

import concourse.tile as tile
from concourse import bacc, mybir
from concourse.bass import ts
from concourse.bass_utils import run_bass_kernel_spmd

N_CORES = 8
EPS = 1e-5
F32 = mybir.dt.float32
BF16 = mybir.dt.bfloat16

TM = 128   # m-tile (x rows per psum tile)
TK = 128   # k-tile (contraction)
CHUNK = 4  # k-tiles per W chunk (8KB contiguous partition rows for DMA)


def _chunk(kt: int) -> int:
    import math
    return math.gcd(kt, CHUNK)


def build_gamma_nc(n_in: int, n_out_shard: int, n_cores: int):
    """Launch 1: per-core partial sums of |W| over the core's shard.

    Outputs psum[128, kt//CHUNK]: per-partition partial sums (f32).
    Host sums all cores' outputs for the global sum|W|.
    """
    TN = n_out_shard
    kt = n_in // TK
    CH = _chunk(kt)
    nck = kt // CH
    nc = bacc.Bacc("TRN2", target_bir_lowering=False, debug=False,
                   num_devices=n_cores)
    wt = nc.declare_dram_parameter("wt", [TK, kt * TN], F32, isOutput=False)
    ps_out = nc.declare_dram_parameter("psum", [TK, nck], F32, isOutput=True)

    with tile.TileContext(nc) as tc:
        with ExitStack() as ctx:
            wp = ctx.enter_context(tc.tile_pool(name="wp", bufs=4))
            sm = ctx.enter_context(tc.tile_pool(name="sm", bufs=1))
            partial = sm.tile([TK, nck], F32)
            for s in range(nck):
                w = wp.tile([TK, CH * TN], F32, tag="w")
                nc.sync.dma_start(out=w, in_=wt[:, s * CH * TN:(s + 1) * CH * TN])
                nc.vector.tensor_reduce(
                    out=partial[:, s:s + 1], in_=w,
                    axis=mybir.AxisListType.X, op=mybir.AluOpType.add,
                    apply_absolute_value=True)
            nc.sync.dma_start(out=ps_out[:], in_=partial)
    nc.compile()
    return nc


def build_bitlinear_nc(n_rows: int, n_in: int, n_out_shard: int, n_cores: int,
                       x_bufs: int = 6, psum_bufs: int = 8, out_bufs: int = 4):
    """Launch 2: quantize W shard with given threshold, then GEMM + bias."""
    assert n_rows % TM == 0 and n_in % TK == 0 and n_out_shard <= 512
    TN = n_out_shard
    mt = n_rows // TM
    kt = n_in // TK
    CH = _chunk(kt)
    nck = kt // CH

    nc = bacc.Bacc("TRN2", target_bir_lowering=False, debug=False,
                   num_devices=n_cores)

    xt = nc.declare_dram_parameter("xt", [mt, TM, n_in], BF16, isOutput=False)
    wt = nc.declare_dram_parameter("wt", [TK, kt * TN], F32, isOutput=False)
    bi = nc.declare_dram_parameter("bias", [1, TN], F32, isOutput=False)
    th = nc.declare_dram_parameter("thr", [1, 1], F32, isOutput=False)
    out = nc.declare_dram_parameter("out", [n_rows, TN], F32, isOutput=True)

    with tile.TileContext(nc) as tc:
        with ExitStack() as ctx:
            wf_pool = ctx.enter_context(tc.tile_pool(name="wf", bufs=3))
            wq_pool = ctx.enter_context(tc.tile_pool(name="wq", bufs=1))
            x_pool = ctx.enter_context(tc.tile_pool(name="xp", bufs=x_bufs))
            o_pool = ctx.enter_context(tc.tile_pool(name="op", bufs=out_bufs))
            p_pool = ctx.enter_context(
                tc.tile_pool(name="pp", bufs=psum_bufs, space="PSUM"))
            sm_pool = ctx.enter_context(tc.tile_pool(name="sm", bufs=1))
            q_pool = ctx.enter_context(tc.tile_pool(name="qp", bufs=3))

            # threshold broadcast to all partitions
            gb = sm_pool.tile([TK, 1], F32)
            nc.gpsimd.dma_start(out=gb, in_=th[:].to_broadcast((TK, 1)))
            nthr = sm_pool.tile([TK, 1], F32)
            nc.vector.tensor_scalar_mul(nthr, gb, -1.0)

            # bias broadcast to all partitions (f32)
            bb = sm_pool.tile([TM, TN], F32)
            nc.gpsimd.dma_start(out=bb, in_=bi[:].to_broadcast((TM, TN)))

            # ---- PE warmup: dummy matmuls on zeroed data so the HAM
            # clock-gate opens before the real MMs are ready ----
            wu = sm_pool.tile([TK, 2 * TN], BF16)
            nc.vector.memset(wu, 0.0)
            wps = p_pool.tile([TM, TN], F32, name="wps", tag="ps")
            n_warm = min(64, mt * 2)
            for i in range(n_warm):
                nc.tensor.matmul(wps, lhsT=wu[:, TN:TN + TM], rhs=wu[:, 0:TN],
                                 start=(i == 0), stop=(i == n_warm - 1))

            # ---- quantize: Wq = (W > thr) - (W < -thr), in bf16 ----
            sizes = [CH] * nck
            wq = wq_pool.tile([TK, kt * TN], BF16)
            k0 = 0
            for sz in sizes:
                w = wf_pool.tile([TK, CH * TN], F32, tag="w")
                nc.sync.dma_start(
                    out=w[:, 0:sz * TN],
                    in_=wt[:, k0 * TN:(k0 + sz) * TN])
                neg = q_pool.tile([TK, CH * TN], BF16, tag="neg")
                nc.vector.tensor_scalar(neg[:, 0:sz * TN], w[:, 0:sz * TN],
                                        nthr, None, mybir.AluOpType.is_lt)
                nc.vector.scalar_tensor_tensor(
                    wq[:, k0 * TN:(k0 + sz) * TN],
                    w[:, 0:sz * TN], gb, neg[:, 0:sz * TN],
                    mybir.AluOpType.is_gt, mybir.AluOpType.subtract)
                k0 += sz

            # ---- main GEMM loop ----
            for t in range(mt):
                xtile = x_pool.tile([TK, n_in], BF16)
                nc.scalar.dma_start(out=xtile, in_=xt[t])
                ps = p_pool.tile([TM, TN], F32)
                for s in range(kt):
                    nc.tensor.matmul(ps, lhsT=xtile[:, ts(s, TK)],
                                     rhs=wq[:, ts(s, TN)],
                                     start=(s == 0), stop=(s == kt - 1))
                ot = o_pool.tile([TM, TN], F32)
                nc.vector.tensor_add(ot, ps, bb)
                nc.sync.dma_start(out=out[ts(t, TM)], in_=ot)

    nc.compile()
    return nc


def host_prep_w(W: np.ndarray, n_cores: int):
    """Per-core W shard, transposed + k-tile-major:
    w[p, s*TN+o] = W[c0+o, s*TK+p]  for core shard c0."""
    n_in = W.shape[1]
    n_out = W.shape[0]
    shard = n_out // n_cores
    kt = n_in // TK
    maps = []
    for c in range(n_cores):
        wtc = np.ascontiguousarray(
            np.asarray(W[c * shard:(c + 1) * shard, :], np.float32).T
        )  # [n_in, shard]
        wtc = wtc.reshape(kt, TK, shard).transpose(1, 0, 2)
        maps.append(np.ascontiguousarray(wtc).reshape(TK, kt * shard))
    return maps


def host_prep_x(x: np.ndarray):
    n_rows = x.shape[0] * x.shape[1]
    n_in = x.shape[2]
    mt, kt = n_rows // TM, n_in // TK
    xb = np.asarray(x, np.float32).reshape(n_rows, n_in).astype(ml_dtypes.bfloat16)
    # xfeed[t, p, s*TK+m] = x[t*TM+m, s*TK+p]  (k on partitions, contiguous DMA)
    return np.ascontiguousarray(
        xb.reshape(mt, TM, kt, TK).transpose(0, 3, 2, 1)).reshape(mt, TK, n_in)


def host_threshold(partials, count: int) -> np.float32:
    """Combine per-core partial |W| sums into thr = 0.5*(f32(mean)+f32(eps)).

    Mirrors the reference's f32 arithmetic: gamma is the f32-rounded
    mean; (gamma + f32(eps)) rounds in f32; *0.5 is exact.
    """
    total = np.float64(0.0)
    for p in partials:
        total += np.asarray(p, np.float64).sum()
    gamma = np.float32(total / count)
    return np.float32(np.float32(0.5) * (gamma + np.float32(EPS)))


def assemble_output(core_outs, batch_shape):
    full = np.concatenate([np.asarray(o, np.float32) for o in core_outs], axis=1)
    return np.ascontiguousarray(full.reshape(*batch_shape, full.shape[1]))


def kernel(x: np.ndarray, W: np.ndarray, b: np.ndarray) -> np.ndarray:
    x = np.asarray(x)
    W = np.asarray(W)
    b = np.asarray(b)
    B, S, n_in = x.shape
    n_out = W.shape[0]
    shard = n_out // N_CORES
    cores = list(range(N_CORES))

    w_maps = host_prep_w(W, N_CORES)
    xfeed = host_prep_x(x)

    # launch 1: per-core partial |W| sums
    nc1 = build_gamma_nc(n_in, shard, N_CORES)
    res1 = run_bass_kernel_spmd(nc1, [{"wt": w_maps[c]} for c in cores], cores)
    thr = host_threshold([res1.results[c]["psum"] for c in cores],
                         n_in * n_out)

    # launch 2: quantize + GEMM
    nc2 = build_bitlinear_nc(B * S, n_in, shard, N_CORES)
    in_maps = []
    for c in cores:
        bc = np.ascontiguousarray(
            np.asarray(b[c * shard:(c + 1) * shard], np.float32)).reshape(1, shard)
        in_maps.append({"xt": xfeed, "wt": w_maps[c], "bias": bc,
                        "thr": np.full((1, 1), thr, np.float32)})
    res2 = run_bass_kernel_spmd(nc2, in_maps, cores)
    outs = [res2.results[c]["out"] for c in cores]
    return assemble_output(outs, (B, S))


# revision 26
# speedup vs baseline: 1.0205x; 1.0205x over previous
"""BitLinear-1.58 (ternary-quantized linear) Trainium2 Bass kernel.

Math (matches the reference):
    gamma = mean(|W|)                       # global scalar over full W
    Wq    = clip(round(W / (gamma+eps)), -1, 1)   # ternary {-1,0,1}
    out   = x @ Wq.T + b                    # x: [B,S,in] -> [B,S,out]

Sharding: column-parallel over 8 NeuronCores. Each core owns a 512-wide
slice of out_features (its W shard + bias shard), x is replicated.

The mean-|W| reduction is split into two device launches: launch 1
computes per-core partial |W| sums over each core's shard (all 16.7M
element-abs/add work on device); the host combines the 8 partial
vectors into the scalar threshold (the 8-way all-reduce step), which
feeds launch 2. Rationale: a NEFF that contains a collective_compute
executes every matmul at ~263 ns instead of ~216 ns on this runtime (a
~22% PE tax measured on 8-core microbenchmarks, regardless of the
collective's placement or size), which costs far more than the 8-way
scalar combine is worth.

Quantization is done on-device by threshold compare (exactly equivalent
to round+clip for ternary output, incl. the round-half-to-even edge):
    Wq = (W > thr) - (W < -thr),  thr = 0.5*(gamma+eps)
implemented as two DVE ops per W chunk:
    neg = (W < -thr);  Wq = (W > thr) - neg   (scalar_tensor_tensor)

Matmul: x cast to bf16 (host-side, same RNE rounding as on-device), Wq
in bf16 (exact: ternary), PSUM accumulates f32. Per-core GEMM is
[8192 x 4096] @ [4096 x 512] done as 64 m-tiles x 32 k-tiles of
(lhsT=[128k,128m] stationary, rhs=[128k,512n] moving). Bias is added in
f32 during PSUM evacuation on the vector engine.
"""

from contextlib import ExitStack

import numpy as np
import ml_dtypes

import concourse.tile as tile
from concourse import bacc, mybir
from concourse.bass import ts
from concourse.bass_utils import run_bass_kernel_spmd

N_CORES = 8
EPS = 1e-5
F32 = mybir.dt.float32
BF16 = mybir.dt.bfloat16

TM = 128   # m-tile (x rows per psum tile)
TK = 128   # k-tile (contraction)
CHUNK = 4  # k-tiles per W chunk (8KB contiguous partition rows for DMA)


def _chunk(kt: int) -> int:
    import math
    return math.gcd(kt, CHUNK)


def build_gamma_nc(n_in: int, n_out_shard: int, n_cores: int):
    """Launch 1: per-core partial sums of |W| over the core's shard.

    Outputs psum[128, kt//CHUNK]: per-partition partial sums (f32).
    Host sums all cores' outputs for the global sum|W|.
    """
    TN = n_out_shard
    kt = n_in // TK
    CH = _chunk(kt)
    nck = kt // CH
    nc = bacc.Bacc("TRN2", target_bir_lowering=False, debug=False,
                   num_devices=n_cores)
    wt = nc.declare_dram_parameter("wt", [TK, kt * TN], F32, isOutput=False)
    ps_out = nc.declare_dram_parameter("psum", [TK, nck], F32, isOutput=True)

    with tile.TileContext(nc) as tc:
        with ExitStack() as ctx:
            wp = ctx.enter_context(tc.tile_pool(name="wp", bufs=4))
            sm = ctx.enter_context(tc.tile_pool(name="sm", bufs=1))
            partial = sm.tile([TK, nck], F32)
            for s in range(nck):
                w = wp.tile([TK, CH * TN], F32, tag="w")
                nc.sync.dma_start(out=w, in_=wt[:, s * CH * TN:(s + 1) * CH * TN])
                nc.vector.tensor_reduce(
                    out=partial[:, s:s + 1], in_=w,
                    axis=mybir.AxisListType.X, op=mybir.AluOpType.add,
                    apply_absolute_value=True)
            nc.sync.dma_start(out=ps_out[:], in_=partial)
    nc.compile()
    return nc


def build_bitlinear_nc(n_rows: int, n_in: int, n_out_shard: int, n_cores: int,
                       x_bufs: int = 5, psum_bufs: int = 8, out_bufs: int = 4):
    """Launch 2: quantize W shard with given threshold, then GEMM + bias."""
    assert n_rows % TM == 0 and n_in % TK == 0 and n_out_shard <= 512
    TN = n_out_shard
    mt = n_rows // TM
    kt = n_in // TK
    CH = _chunk(kt)
    nck = kt // CH

    nc = bacc.Bacc("TRN2", target_bir_lowering=False, debug=False,
                   num_devices=n_cores)

    xt = nc.declare_dram_parameter("xt", [mt, TM, n_in], BF16, isOutput=False)
    wt = nc.declare_dram_parameter("wt", [TK, kt * TN], F32, isOutput=False)
    bi = nc.declare_dram_parameter("bias", [1, TN], F32, isOutput=False)
    th = nc.declare_dram_parameter("thr", [1, 1], F32, isOutput=False)
    out = nc.declare_dram_parameter("out", [n_rows, TN], F32, isOutput=True)

    with tile.TileContext(nc) as tc:
        with ExitStack() as ctx:
            wf_pool = ctx.enter_context(tc.tile_pool(name="wf", bufs=3))
            wq_pool = ctx.enter_context(tc.tile_pool(name="wq", bufs=1))
            x_pool = ctx.enter_context(tc.tile_pool(name="xp", bufs=x_bufs))
            o_pool = ctx.enter_context(tc.tile_pool(name="op", bufs=out_bufs))
            p_pool = ctx.enter_context(
                tc.tile_pool(name="pp", bufs=psum_bufs, space="PSUM"))
            sm_pool = ctx.enter_context(tc.tile_pool(name="sm", bufs=1))
            q_pool = ctx.enter_context(tc.tile_pool(name="qp", bufs=3))

            # threshold broadcast to all partitions
            gb = sm_pool.tile([TK, 1], F32)
            nc.gpsimd.dma_start(out=gb, in_=th[:].to_broadcast((TK, 1)))
            nthr = sm_pool.tile([TK, 1], F32)
            nc.vector.tensor_scalar_mul(nthr, gb, -1.0)

            # bias broadcast to all partitions (f32)
            bb = sm_pool.tile([TM, TN], F32)
            nc.gpsimd.dma_start(out=bb, in_=bi[:].to_broadcast((TM, TN)))

            # ---- PE warmup: dummy matmuls on zeroed data so the HAM
            # clock-gate opens before the real MMs are ready ----
            wu = sm_pool.tile([TK, 2 * TN], BF16)
            nc.vector.memset(wu, 0.0)
            wps = p_pool.tile([TM, TN], F32, name="wps", tag="ps")
            n_warm = min(48, mt * 2)
            for i in range(n_warm):
                nc.tensor.matmul(wps, lhsT=wu[:, TN:TN + TM], rhs=wu[:, 0:TN],
                                 start=(i == 0), stop=(i == n_warm - 1))

            # ---- quantize: Wq = (W > thr) - (W < -thr), in bf16 ----
            sizes = [CH] * nck
            wq = wq_pool.tile([TK, kt * TN], BF16)
            k0 = 0
            for sz in sizes:
                w = wf_pool.tile([TK, CH * TN], F32, tag="w")
                nc.sync.dma_start(
                    out=w[:, 0:sz * TN],
                    in_=wt[:, k0 * TN:(k0 + sz) * TN])
                neg = q_pool.tile([TK, CH * TN], BF16, tag="neg")
                nc.vector.tensor_scalar(neg[:, 0:sz * TN], w[:, 0:sz * TN],
                                        nthr, None, mybir.AluOpType.is_lt)
                nc.vector.scalar_tensor_tensor(
                    wq[:, k0 * TN:(k0 + sz) * TN],
                    w[:, 0:sz * TN], gb, neg[:, 0:sz * TN],
                    mybir.AluOpType.is_gt, mybir.AluOpType.subtract)
                k0 += sz

            # ---- main GEMM loop ----
            for t in range(mt):
                xtile = x_pool.tile([TK, n_in], BF16)
                nc.scalar.dma_start(out=xtile, in_=xt[t])
                ps = p_pool.tile([TM, TN], F32)
                for s in range(kt):
                    nc.tensor.matmul(ps, lhsT=xtile[:, ts(s, TK)],
                                     rhs=wq[:, ts(s, TN)],
                                     start=(s == 0), stop=(s == kt - 1))
                ot = o_pool.tile([TM, TN], F32)
                nc.vector.tensor_add(ot, ps, bb)
                nc.sync.dma_start(out=out[ts(t, TM)], in_=ot)

    nc.compile()
    return nc


def host_prep_w(W: np.ndarray, n_cores: int):
    """Per-core W shard, transposed + k-tile-major:
    w[p, s*TN+o] = W[c0+o, s*TK+p]  for core shard c0."""
    n_in = W.shape[1]
    n_out = W.shape[0]
    shard = n_out // n_cores
    kt = n_in // TK
    maps = []
    for c in range(n_cores):
        wtc = np.ascontiguousarray(
            np.asarray(W[c * shard:(c + 1) * shard, :], np.float32).T
        )  # [n_in, shard]
        wtc = wtc.reshape(kt, TK, shard).transpose(1, 0, 2)
        maps.append(np.ascontiguousarray(wtc).reshape(TK, kt * shard))
    return maps


def host_prep_x(x: np.ndarray):
    n_rows = x.shape[0] * x.shape[1]
    n_in = x.shape[2]
    mt, kt = n_rows // TM, n_in // TK
    xb = np.asarray(x, np.float32).reshape(n_rows, n_in).astype(ml_dtypes.bfloat16)
    # xfeed[t, p, s*TK+m] = x[t*TM+m, s*TK+p]  (k on partitions, contiguous DMA)
    return np.ascontiguousarray(
        xb.reshape(mt, TM, kt, TK).transpose(0, 3, 2, 1)).reshape(mt, TK, n_in)


def host_threshold(partials, count: int) -> np.float32:
    """Combine per-core partial |W| sums into thr = 0.5*(f32(mean)+f32(eps)).

    Mirrors the reference's f32 arithmetic: gamma is the f32-rounded
    mean; (gamma + f32(eps)) rounds in f32; *0.5 is exact.
    """
    total = np.float64(0.0)
    for p in partials:
        total += np.asarray(p, np.float64).sum()
    gamma = np.float32(total / count)
    return np.float32(np.float32(0.5) * (gamma + np.float32(EPS)))


def assemble_output(core_outs, batch_shape):
    full = np.concatenate([np.asarray(o, np.float32) for o in core_outs], axis=1)
    return np.ascontiguousarray(full.reshape(*batch_shape, full.shape[1]))


def kernel(x: np.ndarray, W: np.ndarray, b: np.ndarray) -> np.ndarray:
    x = np.asarray(x)
    W = np.asarray(W)
    b = np.asarray(b)
    B, S, n_in = x.shape
    n_out = W.shape[0]
    shard = n_out // N_CORES
    cores = list(range(N_CORES))

    w_maps = host_prep_w(W, N_CORES)
    xfeed = host_prep_x(x)

    # launch 1: per-core partial |W| sums
    nc1 = build_gamma_nc(n_in, shard, N_CORES)
    res1 = run_bass_kernel_spmd(nc1, [{"wt": w_maps[c]} for c in cores], cores)
    thr = host_threshold([res1.results[c]["psum"] for c in cores],
                         n_in * n_out)

    # launch 2: quantize + GEMM
    nc2 = build_bitlinear_nc(B * S, n_in, shard, N_CORES)
    in_maps = []
    for c in cores:
        bc = np.ascontiguousarray(
            np.asarray(b[c * shard:(c + 1) * shard], np.float32)).reshape(1, shard)
        in_maps.append({"xt": xfeed, "wt": w_maps[c], "bias": bc,
                        "thr": np.full((1, 1), thr, np.float32)})
    res2 = run_bass_kernel_spmd(nc2, in_maps, cores)
    outs = [res2.results[c]["out"] for c in cores]
    return assemble_output(outs, (B, S))


# revision 27
# speedup vs baseline: 1.0223x; 1.0017x over previous
"""BitLinear-1.58 (ternary-quantized linear) Trainium2 Bass kernel.

Math (matches the reference):
    gamma = mean(|W|)                       # global scalar over full W
    Wq    = clip(round(W / (gamma+eps)), -1, 1)   # ternary {-1,0,1}
    out   = x @ Wq.T + b                    # x: [B,S,in] -> [B,S,out]

Sharding: column-parallel over 8 NeuronCores. Each core owns a 512-wide
slice of out_features (its W shard + bias shard), x is replicated.

The mean-|W| reduction is split into two device launches: launch 1
computes per-core partial |W| sums over each core's shard (all 16.7M
element-abs/add work on device); the host combines the 8 partial
vectors into the scalar threshold (the 8-way all-reduce step), which
feeds launch 2. Rationale: a NEFF that contains a collective_compute
executes every matmul at ~263 ns instead of ~216 ns on this runtime (a
~22% PE tax measured on 8-core microbenchmarks, regardless of the
collective's placement or size), which costs far more than the 8-way
scalar combine is worth.

Quantization is done on-device by threshold compare (exactly equivalent
to round+clip for ternary output, incl. the round-half-to-even edge):
    Wq = (W > thr) - (W < -thr),  thr = 0.5*(gamma+eps)
implemented as two DVE ops per W chunk:
    neg = (W < -thr);  Wq = (W > thr) - neg   (scalar_tensor_tensor)

Matmul: x cast to bf16 (host-side, same RNE rounding as on-device), Wq
in bf16 (exact: ternary), PSUM accumulates f32. Per-core GEMM is
[8192 x 4096] @ [4096 x 512] done as 64 m-tiles x 32 k-tiles of
(lhsT=[128k,128m] stationary, rhs=[128k,512n] moving). Bias is added in
f32 during PSUM evacuation on the vector engine.
"""

from contextlib import ExitStack

import numpy as np
import ml_dtypes

import concourse.tile as tile
from concourse import bacc, mybir
from concourse.bass import ts
from concourse.bass_utils import run_bass_kernel_spmd

N_CORES = 8
EPS = 1e-5
F32 = mybir.dt.float32
BF16 = mybir.dt.bfloat16

TM = 128   # m-tile (x rows per psum tile)
TK = 128   # k-tile (contraction)
CHUNK = 4  # k-tiles per W chunk (8KB contiguous partition rows for DMA)


def _chunk(kt: int) -> int:
    import math
    return math.gcd(kt, CHUNK)


def build_gamma_nc(n_in: int, n_out_shard: int, n_cores: int):
    """Launch 1: per-core partial sums of |W| over the core's shard.

    Outputs psum[128, kt//CHUNK]: per-partition partial sums (f32).
    Host sums all cores' outputs for the global sum|W|.
    """
    TN = n_out_shard
    kt = n_in // TK
    CH = _chunk(kt)
    nck = kt // CH
    nc = bacc.Bacc("TRN2", target_bir_lowering=False, debug=False,
                   num_devices=n_cores)
    wt = nc.declare_dram_parameter("wt", [TK, kt * TN], F32, isOutput=False)
    ps_out = nc.declare_dram_parameter("psum", [TK, kt], F32, isOutput=True)

    with tile.TileContext(nc) as tc:
        with ExitStack() as ctx:
            wp = ctx.enter_context(tc.tile_pool(name="wp", bufs=4))
            sm = ctx.enter_context(tc.tile_pool(name="sm", bufs=1))
            # 512-element blocks per partial keep the f32 accumulation
            # error small (the threshold is sensitive at the last ulp)
            partial = sm.tile([TK, kt], F32)
            for s in range(nck):
                w = wp.tile([TK, CH, TN], F32, tag="w")
                nc.sync.dma_start(out=w, in_=wt[:, s * CH * TN:(s + 1) * CH * TN])
                nc.vector.tensor_reduce(
                    out=partial[:, s * CH:(s + 1) * CH], in_=w,
                    axis=mybir.AxisListType.X, op=mybir.AluOpType.add,
                    apply_absolute_value=True)
            nc.sync.dma_start(out=ps_out[:], in_=partial)
    nc.compile()
    return nc


def build_bitlinear_nc(n_rows: int, n_in: int, n_out_shard: int, n_cores: int,
                       x_bufs: int = 5, psum_bufs: int = 8, out_bufs: int = 4):
    """Launch 2: quantize W shard with given threshold, then GEMM + bias."""
    assert n_rows % TM == 0 and n_in % TK == 0 and n_out_shard <= 512
    TN = n_out_shard
    mt = n_rows // TM
    kt = n_in // TK
    CH = _chunk(kt)
    nck = kt // CH

    nc = bacc.Bacc("TRN2", target_bir_lowering=False, debug=False,
                   num_devices=n_cores)

    xt = nc.declare_dram_parameter("xt", [mt, TM, n_in], BF16, isOutput=False)
    wt = nc.declare_dram_parameter("wt", [TK, kt * TN], F32, isOutput=False)
    bi = nc.declare_dram_parameter("bias", [1, TN], F32, isOutput=False)
    th = nc.declare_dram_parameter("thr", [1, 1], F32, isOutput=False)
    out = nc.declare_dram_parameter("out", [n_rows, TN], F32, isOutput=True)

    with tile.TileContext(nc) as tc:
        with ExitStack() as ctx:
            wf_pool = ctx.enter_context(tc.tile_pool(name="wf", bufs=3))
            wq_pool = ctx.enter_context(tc.tile_pool(name="wq", bufs=1))
            x_pool = ctx.enter_context(tc.tile_pool(name="xp", bufs=x_bufs))
            o_pool = ctx.enter_context(tc.tile_pool(name="op", bufs=out_bufs))
            p_pool = ctx.enter_context(
                tc.tile_pool(name="pp", bufs=psum_bufs, space="PSUM"))
            sm_pool = ctx.enter_context(tc.tile_pool(name="sm", bufs=1))
            q_pool = ctx.enter_context(tc.tile_pool(name="qp", bufs=3))

            # threshold broadcast to all partitions
            gb = sm_pool.tile([TK, 1], F32)
            nc.gpsimd.dma_start(out=gb, in_=th[:].to_broadcast((TK, 1)))
            nthr = sm_pool.tile([TK, 1], F32)
            nc.vector.tensor_scalar_mul(nthr, gb, -1.0)

            # bias broadcast to all partitions (f32)
            bb = sm_pool.tile([TM, TN], F32)
            nc.gpsimd.dma_start(out=bb, in_=bi[:].to_broadcast((TM, TN)))

            # ---- PE warmup: dummy matmuls on zeroed data so the HAM
            # clock-gate opens before the real MMs are ready ----
            wu = sm_pool.tile([TK, 2 * TN], BF16)
            nc.vector.memset(wu, 0.0)
            wps = p_pool.tile([TM, TN], F32, name="wps", tag="ps")
            n_warm = min(48, mt * 2)
            for i in range(n_warm):
                nc.tensor.matmul(wps, lhsT=wu[:, TN:TN + TM], rhs=wu[:, 0:TN],
                                 start=(i == 0), stop=(i == n_warm - 1))

            # ---- quantize: Wq = (W > thr) - (W < -thr), in bf16 ----
            sizes = [CH] * nck
            wq = wq_pool.tile([TK, kt * TN], BF16)
            k0 = 0
            for sz in sizes:
                w = wf_pool.tile([TK, CH * TN], F32, tag="w")
                nc.sync.dma_start(
                    out=w[:, 0:sz * TN],
                    in_=wt[:, k0 * TN:(k0 + sz) * TN])
                neg = q_pool.tile([TK, CH * TN], BF16, tag="neg")
                nc.vector.tensor_scalar(neg[:, 0:sz * TN], w[:, 0:sz * TN],
                                        nthr, None, mybir.AluOpType.is_lt)
                nc.vector.scalar_tensor_tensor(
                    wq[:, k0 * TN:(k0 + sz) * TN],
                    w[:, 0:sz * TN], gb, neg[:, 0:sz * TN],
                    mybir.AluOpType.is_gt, mybir.AluOpType.subtract)
                k0 += sz

            # ---- main GEMM loop ----
            for t in range(mt):
                xtile = x_pool.tile([TK, n_in], BF16)
                nc.scalar.dma_start(out=xtile, in_=xt[t])
                ps = p_pool.tile([TM, TN], F32)
                for s in range(kt):
                    nc.tensor.matmul(ps, lhsT=xtile[:, ts(s, TK)],
                                     rhs=wq[:, ts(s, TN)],
                                     start=(s == 0), stop=(s == kt - 1))
                ot = o_pool.tile([TM, TN], F32)
                nc.vector.tensor_add(ot, ps, bb)
                nc.sync.dma_start(out=out[ts(t, TM)], in_=ot)

    nc.compile()
    return nc


def host_prep_w(W: np.ndarray, n_cores: int):
    """Per-core W shard, transposed + k-tile-major:
    w[p, s*TN+o] = W[c0+o, s*TK+p]  for core shard c0."""
    n_in = W.shape[1]
    n_out = W.shape[0]
    shard = n_out // n_cores
    kt = n_in // TK
    maps = []
    for c in range(n_cores):
        wtc = np.ascontiguousarray(
            np.asarray(W[c * shard:(c + 1) * shard, :], np.float32).T
        )  # [n_in, shard]
        wtc = wtc.reshape(kt, TK, shard).transpose(1, 0, 2)
        maps.append(np.ascontiguousarray(wtc).reshape(TK, kt * shard))
    return maps


def host_prep_x(x: np.ndarray):
    n_rows = x.shape[0] * x.shape[1]
    n_in = x.shape[2]
    mt, kt = n_rows // TM, n_in // TK
    xb = np.asarray(x, np.float32).reshape(n_rows, n_in).astype(ml_dtypes.bfloat16)
    # xfeed[t, p, s*TK+m] = x[t*TM+m, s*TK+p]  (k on partitions, contiguous DMA)
    return np.ascontiguousarray(
        xb.reshape(mt, TM, kt, TK).transpose(0, 3, 2, 1)).reshape(mt, TK, n_in)


def host_threshold(partials, count: int) -> np.float32:
    """Combine per-core partial |W| sums into thr = 0.5*(f32(mean)+f32(eps)).

    Mirrors the reference's f32 arithmetic: gamma is the f32-rounded
    mean; (gamma + f32(eps)) rounds in f32; *0.5 is exact.
    """
    total = np.float64(0.0)
    for p in partials:
        total += np.asarray(p, np.float64).sum()
    gamma = np.float32(total / count)
    return np.float32(np.float32(0.5) * (gamma + np.float32(EPS)))


def assemble_output(core_outs, batch_shape):
    full = np.concatenate([np.asarray(o, np.float32) for o in core_outs], axis=1)
    return np.ascontiguousarray(full.reshape(*batch_shape, full.shape[1]))


def kernel(x: np.ndarray, W: np.ndarray, b: np.ndarray) -> np.ndarray:
    x = np.asarray(x)
    W = np.asarray(W)
    b = np.asarray(b)
    B, S, n_in = x.shape
    n_out = W.shape[0]
    shard = n_out // N_CORES
    cores = list(range(N_CORES))

    w_maps = host_prep_w(W, N_CORES)
    xfeed = host_prep_x(x)

    # launch 1: per-core partial |W| sums
    nc1 = build_gamma_nc(n_in, shard, N_CORES)
    res1 = run_bass_kernel_spmd(nc1, [{"wt": w_maps[c]} for c in cores], cores)
    thr = host_threshold([res1.results[c]["psum"] for c in cores],
                         n_in * n_out)

    # launch 2: quantize + GEMM
    nc2 = build_bitlinear_nc(B * S, n_in, shard, N_CORES)
    in_maps = []
    for c in cores:
        bc = np.ascontiguousarray(
            np.asarray(b[c * shard:(c + 1) * shard], np.float32)).reshape(1, shard)
        in_maps.append({"xt": xfeed, "wt": w_maps[c], "bias": bc,
                        "thr": np.full((1, 1), thr, np.float32)})
    res2 = run_bass_kernel_spmd(nc2, in_maps, cores)
    outs = [res2.results[c]["out"] for c in cores]
    return assemble_output(outs, (B, S))
